# revision 4
# baseline (speedup 1.0000x reference)
"""Causal self-attention (B=2, S=2048, D=1024, H=16) on 8 TRN2 NeuronCores.

Sharding: core c -> batch b = c//4, head group g = c%4 (4 heads each).
Each core computes the qkv projection for its heads, RoPE, causal flash
attention, and a partial out-projection (row-parallel); the host sums the
4 partials per batch.

Layout strategy (everything "transposed", seq on the free axis):
  xt  = x[b]^T                  [D, S]   bf16 (host-prepped)
  Qt/Kt[m, s] per head          computed as  W[:,m]^T @ xt  (lhsT = W slice)
  V natural [s, m]              computed as  xt_tile^T @ Wv
  St[k, q]  = Kt_tile^T @ Qt    -> exp -> causal mask -> Pt (bf16)
  Ot'[m+1, q] = [V|1]^T @ Pt    (row m==HD is the softmax denominator l[q])
  y^T[n, s] = Wo[:,n]^T @ (Ot/l)  accumulated over m tiles; host sums cores.

v4 structure (vs v3's 168us):
 - exp on the ACT engine is the largest serial resource (~84us); v3
   back-loaded it (chunk ci has ci+1 key tiles).  v4 emits attention
   pairs in order (2,0)(2,1)(3,0)(3,1)(1,*)(0,*) so the big chunks' exp
   starts ~12us in, and paces emission with a fine-grained scheduler:
   each QK+exp unit is followed by ~0.7us of PE filler (qkv projection
   mt-tiles, V sl-tiles, AV segments of the previous pair, out-proj
   halves) popped from a dependency-tagged queue.
 - causal mask: only the 128-col diagonal window of each tile needs
   masking; one [128,2,128] bf16 mul per diagonal tile (both heads at
   once) instead of the full-tail mul (saves ~25us of Vector).
 - epilogue: 1/l via reciprocal on the PSUM row directly, GpSimd
   broadcast, single fused (O * 1/l) mul from PSUM (saves the osb/l
   copies, ~22us of Vector).
 - inputs split across both HWDGE rings (sync + scalar), ordered so
   kt0/qt2/csn land first; 64 small warm matmuls un-throttle the PE
   HAM clock before the first real matmul.
"""

from contextlib import ExitStack

import numpy as np
import ml_dtypes

import concourse.bass as bass
import concourse.tile as tile
import concourse.mybir as mybir
from concourse import bacc
from concourse.bass_utils import run_bass_kernel_spmd

HD = 64          # head dim
CH = 512         # seq chunk (one PSUM bank of fp32)
_SHUF = [(i + 16) % 32 for i in range(32)]  # swap 16-halves in each quadrant


def rope_perm():
    """Within-head output-column permutation: local row r <- reference col."""
    perm = np.zeros(HD, dtype=np.int64)
    for r in range(HD):
        q, pos = divmod(r, 32)
        x2 = pos >= 16
        f = q * 16 + (pos % 16)
        perm[r] = 2 * f + (1 if x2 else 0)
    return perm


def rope_tables(rope_cos, rope_sin, S):
    """cos/sin tables [128, S] fp32 aligned with the permuted Qt/Kt rows."""
    cs = np.zeros((128, S), np.float32)
    sn = np.zeros((128, S), np.float32)
    for r in range(128):
        rr = r % HD
        q, pos = divmod(rr, 32)
        x2 = pos >= 16
        f = q * 16 + (pos % 16)
        cs[r] = rope_cos[:S, f]
        sn[r] = rope_sin[:S, f] * (-1.0 if x2 else 1.0)
    return cs, sn


def build_core(nc, S, D, HC):
    """Emit the per-core kernel IR. HC = heads on this core."""
    DT = D // 128           # contraction tiles over model dim
    M = HC * HD             # local qkv width
    MT = M // 128           # m tiles
    NCH = S // CH           # seq chunks
    KPC = CH // 128         # key tiles per chunk
    NT = D // 128           # out-proj n tiles
    NT2 = NT // 2
    HP = HC // 2            # head pairs
    fp32, bf16 = mybir.dt.float32, mybir.dt.bfloat16
    SCALE = float(HD) ** -0.5

    xt_d = nc.declare_dram_parameter("xt", [NCH, 128, DT * CH], bf16, isOutput=False)
    wq_d = nc.declare_dram_parameter("wq", [128, DT * M], bf16, isOutput=False)
    wk_d = nc.declare_dram_parameter("wk", [128, DT * M], bf16, isOutput=False)
    wv_d = nc.declare_dram_parameter("wv", [128, DT * M], bf16, isOutput=False)
    wo_d = nc.declare_dram_parameter("wo", [128, MT * D], bf16, isOutput=False)
    csn_d = nc.declare_dram_parameter("csn", [128, 2 * NCH * CH], bf16, isOutput=False)
    yt_d = nc.declare_dram_parameter("yt", [NCH, 2, 128, NT2 * CH], bf16, isOutput=True)

    with tile.TileContext(nc) as tc, ExitStack() as ctx:
        persist = ctx.enter_context(tc.tile_pool(name="persist", bufs=1))
        mm_ps = ctx.enter_context(tc.tile_pool(name="mm_ps", bufs=2, space="PSUM"))
        st_ps = ctx.enter_context(tc.tile_pool(name="st_ps", bufs=2, space="PSUM"))
        ot_ps = ctx.enter_context(tc.tile_pool(name="ot_ps", bufs=2, space="PSUM"))
        work = ctx.enter_context(tc.tile_pool(name="work", bufs=3))
        pt_pool = ctx.enter_context(tc.tile_pool(name="ptp", bufs=26))
        out_pool = ctx.enter_context(tc.tile_pool(name="outp", bufs=2))

        # ---- persistent tiles -------------------------------------------
        xt = [persist.tile([128, DT, CH], bf16, name=f"xt_{c}") for c in range(NCH)]
        wq = persist.tile([128, DT, M], bf16)
        wk = persist.tile([128, DT, M], bf16)
        wv = persist.tile([128, DT, M], bf16)
        wo = persist.tile([128, MT, D], bf16)
        csn = persist.tile([128, 2, NCH, CH], bf16)
        qt = [persist.tile([128, MT, CH], bf16, name=f"qt_{c}") for c in range(NCH)]
        kt = [persist.tile([128, MT, CH], bf16, name=f"kt_{c}") for c in range(NCH)]
        vsb = [persist.tile([128, KPC, HC, HD + 1], bf16, name=f"vsb_{c}")
               for c in range(NCH)]
        otn = [persist.tile([128, MT, CH], bf16, name=f"otn_{c}") for c in range(NCH)]
        cmask = persist.tile([128, 2, 128], bf16)

        # ---- input DMAs split across the two HWDGE rings ----------------
        # ring1 (sync): wk, xt0, xt2, xt1, xt3, wo -- the kt0/qt2 critical
        # path first.  ring2 (scalar): csn (rope needs it first), wq, wv.
        nc.sync.dma_start(out=wk.rearrange("p t m -> p (t m)"), in_=wk_d[:, :])
        nc.scalar.dma_start(
            out=csn.rearrange("p i c s -> p (i c s)"), in_=csn_d[:, :])
        nc.sync.dma_start(
            out=xt[0].rearrange("p t s -> p (t s)"), in_=xt_d[0])
        nc.scalar.dma_start(out=wq.rearrange("p t m -> p (t m)"), in_=wq_d[:, :])
        nc.sync.dma_start(
            out=xt[2].rearrange("p t s -> p (t s)"), in_=xt_d[2])
        nc.scalar.dma_start(out=wv.rearrange("p t m -> p (t m)"), in_=wv_d[:, :])
        nc.sync.dma_start(
            out=xt[1].rearrange("p t s -> p (t s)"), in_=xt_d[1])
        nc.sync.dma_start(
            out=xt[3].rearrange("p t s -> p (t s)"), in_=xt_d[3])
        nc.sync.dma_start(out=wo.rearrange("p t n -> p (t n)"), in_=wo_d[:, :])

        # ---- PE warmup: ~5us of small matmuls un-throttle the HAM clock
        # while the DMAs stream (they depend only on the memsets).
        warm_w = persist.tile([128, 128], bf16)
        nc.vector.memset(warm_w[:], 0.0)
        warm_ps = mm_ps.tile([128, CH], fp32, tag="mm", name="mmps")
        for _ in range(64):
            nc.tensor.matmul(warm_ps[:, 0:128], warm_w[:], warm_w[:],
                             start=True, stop=True)

        # causal mask for the 128-col diagonal window (both head slots):
        # keep j - p >= 0 (query-local j, key-local p).  Same for every
        # diagonal tile.  Built on GpSimd while it is otherwise idle.
        nc.gpsimd.memset(cmask[:], 1.0)
        for i in range(2):
            nc.gpsimd.affine_select(
                out=cmask[:, i, :], in_=cmask[:, i, :],
                compare_op=mybir.AluOpType.is_ge, fill=0.0,
                base=0, pattern=[[1, 128]], channel_multiplier=-1,
            )
        for c in range(NCH):
            nc.vector.memset(vsb[c][:, :, :, HD:HD + 1], 1.0)

        # ---- emission units ---------------------------------------------
        def qk_mt(ci, wt, dst, mt):
            """One mt-tile of a q/k projection + rope."""
            ps = mm_ps.tile([128, CH], fp32, tag="mm", name="mmps")
            for dt in range(DT):
                nc.tensor.matmul(
                    ps[:],
                    wt[:, dt, mt * 128:(mt + 1) * 128],
                    xt[ci][:, dt, :],
                    start=(dt == 0), stop=(dt == DT - 1),
                )
            p1 = work.tile([128, CH], fp32, tag="p1")
            p2 = work.tile([128, CH], fp32, tag="p2")
            p2s = work.tile([128, CH], fp32, tag="p2s")
            nc.vector.tensor_mul(p1[:], ps[:], csn[:, 0, ci, :])
            nc.vector.tensor_mul(p2[:], ps[:], csn[:, 1, ci, :])
            nc.vector.stream_shuffle(p2s[:], p2[:], mask=_SHUF)
            nc.vector.tensor_add(dst[ci][:, mt, :], p1[:], p2s[:])

        def v_sl(ci, sl):
            """One 128-row slice of the V projection."""
            ps = mm_ps.tile([128, M], fp32, tag="mm", name="vps")
            for dt in range(DT):
                nc.tensor.matmul(
                    ps[:],
                    xt[ci][:, dt, sl * 128:(sl + 1) * 128],
                    wv[:, dt, :],
                    start=(dt == 0), stop=(dt == DT - 1),
                )
            nc.vector.tensor_copy(
                vsb[ci][:, sl, :, 0:HD],
                ps.rearrange("p (h d) -> p h d", h=HC),
            )

        pts_map = {}     # (ci, hp) -> list of (pt, trim)
        ots_map = {}     # (ci, hp) -> {h: psum tile}

        def qk_kj(ci, hp, kj):
            """QK scores for one key tile (both heads), exp, diag mask."""
            heads = (2 * hp, 2 * hp + 1)
            mt = hp
            tidx = kj - ci * KPC
            trim = max(0, tidx) * 128
            kc, kl = divmod(kj, KPC)
            stp = st_ps.tile([128, 2, CH], fp32, tag="st")
            for i, h in enumerate(heads):
                base = (h % 2) * 64
                nc.tensor.matmul(
                    stp[:, i, trim:],
                    kt[kc][base:base + HD, mt, kl * 128:(kl + 1) * 128],
                    qt[ci][base:base + HD, mt, trim:],
                    start=True, stop=True,
                )
            pt = pt_pool.tile([128, 2, CH], bf16, tag="pt")
            nc.scalar.activation(
                out=pt[:, :, trim:], in_=stp[:, :, trim:],
                func=mybir.ActivationFunctionType.Exp, scale=SCALE,
            )
            if tidx >= 0:
                nc.vector.tensor_mul(
                    pt[:, :, trim:trim + 128], pt[:, :, trim:trim + 128],
                    cmask[:, :, :],
                )
            pts_map[(ci, hp)].append((pt, trim))

        def av_seg(ci, hp, h, kj0, kj1, nkt):
            """AV accumulation for head h over key tiles [kj0, kj1)."""
            i = h % 2
            ot = ots_map[(ci, hp)][h]
            pts = pts_map[(ci, hp)]
            for kj in range(kj0, kj1):
                pt, trim = pts[kj]
                kc, kl = divmod(kj, KPC)
                nc.tensor.matmul(
                    ot[0:HD + 1, trim:],
                    vsb[kc][:, kl, h, :],
                    pt[:, i, trim:],
                    start=(kj == 0), stop=(kj == nkt - 1),
                )

        def epi(ci, hp, h):
            """1/l scale of the AV output into otn (no copies)."""
            base = (h % 2) * 64
            mt = hp
            ot = ots_map[(ci, hp)][h]
            l_sb = work.tile([1, CH], fp32, tag="l")
            nc.vector.tensor_copy(l_sb[:], ot[HD:HD + 1, :])
            rl = work.tile([1, CH], fp32, tag="rl")
            nc.vector.reciprocal_approx_fast(rl[:], l_sb[:])
            lb = work.tile([64, CH], fp32, tag="lb")
            nc.gpsimd.partition_broadcast(lb[:], rl[0:1, :])
            nc.vector.tensor_mul(
                otn[ci][base:base + HD, mt, :], ot[0:HD, :], lb[:],
            )

        yts = {}

        def proj_half(ci, half, last):
            if half == 0:
                yts[ci] = out_pool.tile([128, NT, CH], bf16, tag="yt", name="yt")
            for nt in range(half * NT2, (half + 1) * NT2):
                ps = mm_ps.tile([128, CH], fp32, tag="mm", name="mmps")
                for mt2 in range(MT):
                    nc.tensor.matmul(
                        ps[:],
                        wo[:, mt2, nt * 128:(nt + 1) * 128],
                        otn[ci][:, mt2, :],
                        start=(mt2 == 0), stop=(mt2 == MT - 1),
                    )
                if last:
                    nc.scalar.copy(yts[ci][:, nt, :], ps[:])
                else:
                    nc.vector.tensor_copy(yts[ci][:, nt, :], ps[:])
            nc.sync.dma_start(
                out=yt_d[ci, half],
                in_=yts[ci][:, half * NT2:(half + 1) * NT2, :]
                .rearrange("p t s -> p (t s)"),
            )

        # ---- dependency-tagged filler scheduler -------------------------
        # Each filler unit: (key, cost_ns, fn).  qk_kj units are emitted at
        # a fixed cadence; after each, ~FILL ns of filler is popped.  Units
        # a qk_kj/av_seg depends on (emission order = engine FIFO order)
        # are force-emitted first.
        filler = []
        emitted = set()

        def emit_unit(idx):
            key, cost, fn = filler.pop(idx)
            emitted.add(key)
            fn()
            return cost

        def force(key):
            for idx, u in enumerate(filler):
                if u[0] == key:
                    emit_unit(idx)
                    return

        def pop_filler(budget):
            while filler and budget > 0:
                budget -= emit_unit(0)

        def push_back(units):
            filler.extend(units)

        def push_front(units):
            filler[0:0] = units

        # pre-phase: kt0/qt2 mt0 first so pair (2,0) can start.
        qk_mt(0, wk, kt, 0)
        qk_mt(2, wq, qt, 0)
        qk_mt(0, wk, kt, 1)
        qk_mt(2, wq, qt, 1)
        emitted |= {('k', 0, 0), ('k', 0, 1), ('q', 2, 0), ('q', 2, 1)}

        KQ_COST = 1750
        V_COST = 900
        AV_COST = 900
        PROJ_COST = 1800
        FILL = 750

        for c in range(NCH):
            if ('k', c, 0) not in emitted:
                for mt in range(MT):
                    filler.append((('k', c, mt), KQ_COST,
                                   lambda c=c, mt=mt: qk_mt(c, wk, kt, mt)))
        for sl in range(KPC):
            filler.append((('v', 0, sl), V_COST, lambda sl=sl: v_sl(0, sl)))
        for sl in range(KPC):
            filler.append((('v', 1, sl), V_COST, lambda sl=sl: v_sl(1, sl)))
        for sl in range(KPC):
            filler.append((('v', 2, sl), V_COST, lambda sl=sl: v_sl(2, sl)))
        for sl in range(KPC):
            filler.append((('v', 3, sl), V_COST, lambda sl=sl: v_sl(3, sl)))
        for c in (3, 1, 0):
            for mt in range(MT):
                filler.append((('q', c, mt), KQ_COST,
                               lambda c=c, mt=mt: qk_mt(c, wq, qt, mt)))

        pair_order = [(2, 0), (2, 1), (3, 0), (3, 1),
                      (1, 0), (1, 1), (0, 0), (0, 1)]

        for (ci, hp) in pair_order:
            nkt = (ci + 1) * KPC
            pts_map[(ci, hp)] = []
            force(('q', ci, hp))
            for kj in range(nkt):
                kc = kj // KPC
                for mt in range(MT):
                    force(('k', kc, mt))
                qk_kj(ci, hp, kj)
                pop_filler(FILL)
            # AV of this pair: needs every vsb[kc<=ci]; force those, then
            # queue the AV segments + epilogues at the front.
            for c in range(ci + 1):
                for sl in range(KPC):
                    force(('v', c, sl))
            # The ot PSUM pool holds one pair (2 tiles); this pair's AV
            # recycles the previous pair's banks, so every earlier pair's
            # AV/epilogue must be emitted first (else the PE FIFO blocks
            # on - or clobbers - an unread denominator row).
            for key in [u[0] for u in list(filler)]:
                if key[0] in ('av', 'epi'):
                    force(key)
            ots_map[(ci, hp)] = {
                h: ot_ps.tile([128, CH], fp32, tag="ot", name=f"ot_{h}")
                for h in (2 * hp, 2 * hp + 1)
            }
            av_units = []
            for h in (2 * hp, 2 * hp + 1):
                for kj0 in range(0, nkt, KPC):
                    kj1 = min(kj0 + KPC, nkt)
                    av_units.append(
                        ((('av', ci, hp, h, kj0)), AV_COST,
                         lambda ci=ci, hp=hp, h=h, kj0=kj0, kj1=kj1, nkt=nkt:
                         av_seg(ci, hp, h, kj0, kj1, nkt)))
                av_units.append(((('epi', ci, hp, h)), 100,
                                 lambda ci=ci, hp=hp, h=h: epi(ci, hp, h)))
            push_front(av_units)
            if hp == HP - 1:
                # out-projection of this chunk once both pairs' epilogues
                # are queued (they precede it in the filler).
                last = (ci, hp) == pair_order[-1]
                push_back([
                    (('proj', ci, 0), PROJ_COST,
                     lambda ci=ci, last=last: proj_half(ci, 0, last)),
                    (('proj', ci, 1), PROJ_COST,
                     lambda ci=ci, last=last: proj_half(ci, 1, last)),
                ])

        pop_filler(10**9)   # drain remaining units


_CACHE = {}


def _get_nc(S, D, HC):
    key = (S, D, HC)
    if key not in _CACHE:
        nc = bacc.Bacc(None, target_bir_lowering=False)
        build_core(nc, S, D, HC)
        nc.compile()
        _CACHE[key] = nc
    return _CACHE[key]


def make_in_maps(x, rope_cos, rope_sin, W_qkv, W_out, n_cores=8):
    B, S, D = x.shape
    H = 16
    groups = n_cores // B          # head groups per batch
    HC = H // groups               # heads per core
    M = HC * HD
    MT = M // 128
    DT, NCH = D // 128, S // CH
    perm = rope_perm()
    bf16 = ml_dtypes.bfloat16
    cs, sn = rope_tables(np.asarray(rope_cos), np.asarray(rope_sin), S)
    csn = np.stack([cs.reshape(128, NCH * CH), sn.reshape(128, NCH * CH)],
                   axis=1).reshape(128, 2 * NCH * CH)
    in_maps = []
    xtb_cache = {}
    for c in range(n_cores):
        b, g = divmod(c, groups)
        heads = np.arange(g * HC, (g + 1) * HC)
        qcols = np.concatenate([h * HD + perm for h in heads])
        vcols = np.concatenate([2 * D + h * HD + np.arange(HD) for h in heads])
        if b not in xtb_cache:
            xtb_cache[b] = np.ascontiguousarray(
                np.asarray(x[b]).T.reshape(DT, 128, NCH, CH)
                .transpose(2, 1, 0, 3).reshape(NCH, 128, DT * CH)
            ).astype(bf16)

        def wfmt(wcols):
            return np.ascontiguousarray(
                wcols.reshape(DT, 128, M).transpose(1, 0, 2).reshape(128, DT * M)
            ).astype(bf16)

        wo_np = np.ascontiguousarray(
            W_out[g * M:(g + 1) * M, :].reshape(MT, 128, D)
            .transpose(1, 0, 2).reshape(128, MT * D)).astype(bf16)
        in_maps.append({
            "xt": xtb_cache[b],
            "wq": wfmt(W_qkv[:, qcols]),
            "wk": wfmt(W_qkv[:, D + qcols]),
            "wv": wfmt(W_qkv[:, vcols]),
            "wo": wo_np,
            "csn": np.ascontiguousarray(csn).astype(bf16),
        })
    return in_maps


def unshard_out(res, B, S, D, n_cores=8):
    NCH, NT = S // CH, D // 128
    NT2 = NT // 2
    out = np.zeros((B, S, D), np.float32)
    for c in range(n_cores):
        yt = res.results[c]["yt"].astype(np.float32)  # [NCH, 2, 128, NT2*CH]
        ytf = (yt.reshape(NCH, 2, 128, NT2, CH)
               .transpose(1, 3, 2, 0, 4).reshape(D, S))
        out[c // (n_cores // B)] += ytf.T
    return out


def kernel(x, rope_cos, rope_sin, W_qkv, W_out):
    x = np.asarray(x)
    W_qkv = np.asarray(W_qkv)
    W_out = np.asarray(W_out)
    B, S, D = x.shape
    n_cores = 8
    HC = 16 // (n_cores // B)
    in_maps = make_in_maps(x, rope_cos, rope_sin, W_qkv, W_out, n_cores)
    nc = _get_nc(S, D, HC)
    res = run_bass_kernel_spmd(nc, in_maps, list(range(n_cores)))
    return unshard_out(res, B, S, D, n_cores)
